# revision 24
# baseline (speedup 1.0000x reference)
"""Trainium2 (8-core SPMD) kernel for the ActorCriticTensorNet MPS head.

reference:
    env0 = einsum('e,eoij->oij', x[0], mps[0])
    for a in 1..63: env = sigmoid(env @ einsum('e,eoij->oij', x[a], mps[a]))
    out = einsum('oii->o', env)

Strategy: the computation factorizes perfectly over the output channel o —
the per-agent contractions mat[a][o] and the 63-step sigmoid chain for
channel o never touch any other channel; the channels only meet in the final
trace vector.  So shard by o: core c receives mps[:, :, c, :, :] (32 MB in
bf16) plus the full (tiny) x, computes all 64 mat[g][c] slices locally,
runs its own o=c chain locally, and writes the single scalar out[c]; the
host assembles the 8 scalars.  Zero inter-core communication, and the chain
consumes mats as phase 1 produces them, so its serial latency hides entirely
under the tensor streaming.

Phase-1 layout (per 8-agent block): weight column (gi, jl) reads the
stride-8 single-free-dim AP f = gi*1024 + 8p + jl (p = 4k + j_hi), giving
psum[p, gi*8+jl] = mat[gi][k, j] (k = p>>2, j = (p&3)*8 + jl).  The
psum->stage copy permutes columns (gi,jl)->(jl,gi) so the DRAM block is
d = 64p + jl*8 + gi = k*256 + j_hi*64 + jl*8 + gi: the store, the one
chain-block load (32 x 256, 512B runs) and the per-agent chain weight views
(stride-8, merging to a single free dim) are all clean APs.

Host-side prep packs shards as mps[c] -> (2, 128, 64*1024) [e-chunk, e_low,
(agent, f)] and x -> (2, 128, 64), so every device DMA is contiguous 2 MB.
All inputs are converted to bf16 on the host (PSUM accumulation is fp32;
measured end-to-end relative error ~6e-4 vs the fp32 reference).
"""

import numpy as np

A, E, O, C = 64, 256, 8, 32
FO = C * C  # per-o mat size: 1024
N_CORES = 8
# phase-1/chain segments (start_agent, n_agents): small at the head so the
# PE starts early, small at the tail so the final chain steps hand off at
# fine granularity, big in the middle for DMA efficiency.
SEGS = [(0, 4), (4, 4)] + [(8 * b, 8) for b in range(1, 8)]
HSEG = 4  # stage/chain handoff granularity (agents)

_CACHE = {}


def _build(debug_out=False):
    from concourse import bacc, mybir, tile
    from concourse.masks import make_identity

    F32 = mybir.dt.float32
    BF16 = mybir.dt.bfloat16
    SIG = mybir.ActivationFunctionType.Sigmoid
    nc = bacc.Bacc(
        "TRN2", target_bir_lowering=False, debug=False, num_devices=N_CORES
    )
    x_d = nc.dram_tensor("inputs", [2, 128, A], BF16, kind="ExternalInput")
    mps_d = nc.dram_tensor(
        "mps", [2, 128, A * FO], BF16, kind="ExternalInput"
    )
    out_d = nc.dram_tensor("out", [1, 1], F32, kind="ExternalOutput")
    if debug_out:
        matdbg_d = nc.dram_tensor("matdbg", [A, FO], BF16, kind="ExternalOutput")
        envdbg_d = nc.dram_tensor("envdbg", [C, C], F32, kind="ExternalOutput")

    with tile.TileContext(nc) as tc:
        with (
            tc.tile_pool(name="dram", bufs=1, space="DRAM") as dram,
            tc.tile_pool(name="mps_pool", bufs=9) as mps_pool,
            tc.tile_pool(name="small", bufs=1) as small,
            tc.tile_pool(name="stage_pool", bufs=4) as stage_pool,
            tc.tile_pool(name="chain_pool", bufs=6) as chain_pool,
            tc.tile_pool(name="env_pool", bufs=4) as env_pool,
            tc.tile_pool(name="ps_mat", bufs=4, space="PSUM") as ps_mat,
            tc.tile_pool(name="ps_chain", bufs=3, space="PSUM") as ps_chain,
            tc.tile_pool(name="ps_misc", bufs=1, space="PSUM") as ps_misc,
        ):
            NH = A // HSEG  # number of half-segments
            mat_drams = [
                dram.tile([1, 128 * 8 * HSEG], BF16, name=f"matd{hi}")
                for hi in range(NH)
            ]

            # x_sb[e_lo, eh*64 + g] = x[g, eh*128 + e_lo]
            x_sb = small.tile([128, 2 * A], BF16)
            for eh in range(2):
                nc.sync.dma_start(x_sb[:, eh * A : (eh + 1) * A], x_d[eh])

            ident = small.tile([C, C], BF16)
            make_identity(nc, ident[:])

            envs = [None]


            cvs = {}

            def chain_prefetch(hi):
                # one clean (32, 32*HSEG) load per half-segment; agent gi's
                # weight view [jh: step 8H x4][jl: step H x8] opt-merges
                # into a single free dim (stride HSEG).
                cb = chain_pool.tile(
                    [C, 32 * HSEG], BF16, tag="chain", name=f"cb{hi}"
                )
                nc.gpsimd.dma_start(
                    cb[:],
                    mat_drams[hi][:].rearrange(
                        "u (k r) -> (u k) r", k=C, r=32 * HSEG
                    ),
                )
                cvs[hi] = cb[:].rearrange(
                    "k (jh jl gi) -> k jh jl gi", jh=4, jl=8, gi=HSEG
                )

            def chain_step(g):
                hi = g // HSEG
                cv, gi = cvs[hi], g % HSEG
                init = g == 0
                ps_g = ps_chain.tile([C, C], BF16 if init else F32, tag="cps")
                if init:
                    nc.tensor.transpose(ps_g[:], cv[:, :, :, gi], ident[:])
                else:
                    nc.tensor.matmul(
                        ps_g[:],
                        cv[:, :, :, gi],
                        envs[0][:],
                        start=True,
                        stop=True,
                    )
                env2 = env_pool.tile([C, C], BF16, tag="env")
                if init:
                    nc.vector.tensor_copy(env2[:], ps_g[:])
                else:
                    nc.scalar.activation(env2[:], ps_g[:], SIG)
                envs[0] = env2

            # Software-pipelined emission: mps DMA in 8-agent (2x2MB)
            # segments for bandwidth; stage->DRAM->chain handoff per
            # 4-agent half-segment so chain steps lag only ~LAG agents.
            LAG_AGENTS = 4
            next_chain = 0
            done_half = [False] * (A // HSEG)

            def flush_half(psum_b, g0, hoff):
                # drain psum columns for agents [g0+hoff, g0+hoff+HSEG)
                hi = (g0 + hoff) // HSEG
                stage = stage_pool.tile([128, 8 * HSEG], BF16, tag="stage")
                nc.vector.tensor_copy(
                    stage[:].rearrange("p (jl gi) -> p jl gi", jl=8, gi=HSEG),
                    psum_b[:, hoff * 8 : (hoff + HSEG) * 8].rearrange(
                        "p (gi jl) -> p jl gi", gi=HSEG, jl=8
                    ),
                )
                nc.gpsimd.dma_start(mat_drams[hi][:], stage[:])
                chain_prefetch(hi)
                done_half[hi] = True

            for si, (g0, w) in enumerate(SEGS):
                psum_b = ps_mat.tile([128, 8 * w], F32, tag="psa")
                tvs = []
                for eh in range(2):
                    t = mps_pool.tile([128, w * FO], BF16, tag="mps")
                    nc.sync.dma_start(
                        t[:], mps_d[eh, :, g0 * FO : (g0 + w) * FO]
                    )
                    tvs.append(
                        t[:].rearrange(
                            "e (gi p jl) -> e gi p jl", gi=w, p=128, jl=8
                        )
                    )
                for gi in range(w):
                    g = g0 + gi
                    for jl in range(8):
                        for eh in range(2):
                            nc.tensor.matmul(
                                psum_b[:, gi * 8 + jl : gi * 8 + jl + 1],
                                tvs[eh][:, gi, :, jl],
                                x_sb[:, eh * A + g : eh * A + g + 1],
                                start=(eh == 0),
                                stop=(eh == 1),
                            )
                    if (gi + 1) % HSEG == 0:
                        flush_half(psum_b, g0, gi + 1 - HSEG)
                    while (
                        next_chain + LAG_AGENTS <= g
                        and done_half[next_chain // HSEG]
                    ):
                        chain_step(next_chain)
                        next_chain += 1
            while next_chain < A:
                chain_step(next_chain)
                next_chain += 1

            env = envs[0]
            if debug_out:
                dbg = small.tile([C, C], F32)
                nc.vector.tensor_copy(dbg[:], env[:])
                nc.sync.dma_start(envdbg_d[:], dbg[:])


            # ---- trace epilogue: out = sum_k env_T[k, k] ----
            masked = small.tile([C, C], BF16)
            nc.gpsimd.affine_select(
                out=masked[:],
                in_=env[:],
                compare_op=mybir.AluOpType.is_equal,
                fill=0.0,
                base=0,
                pattern=[[1, C]],
                channel_multiplier=-1,
            )
            red = small.tile([C, 1], F32)
            nc.vector.tensor_reduce(
                red[:],
                masked[:],
                axis=mybir.AxisListType.X,
                op=mybir.AluOpType.add,
            )
            ones = small.tile([C, 1], F32)
            nc.gpsimd.memset(ones[:], 1.0)
            pt = ps_misc.tile([1, 1], F32)
            nc.tensor.matmul(pt[:], red[:], ones[:], start=True, stop=True)
            osb = small.tile([1, 1], F32)
            nc.vector.tensor_copy(osb[:], pt[:])
            nc.sync.dma_start(out_d[:], osb[:])

    nc.compile()
    return nc


def get_nc():
    if "nc" not in _CACHE:
        _CACHE["nc"] = _build()
    return _CACHE["nc"]


def make_in_maps(inputs, mps):
    import ml_dtypes

    x = np.asarray(inputs, dtype=np.float32).astype(ml_dtypes.bfloat16)
    mps = np.asarray(mps, dtype=np.float32).reshape(A, E, O, FO)
    # x packed as [e_chunk, e_low, agent]
    x_pack = np.ascontiguousarray(x.reshape(A, 2, 128).transpose(1, 2, 0))
    in_maps = []
    for c in range(N_CORES):
        m = mps[:, :, c, :].astype(ml_dtypes.bfloat16)  # (A, E, FO)
        m = m.reshape(A, 2, 128, FO).transpose(1, 2, 0, 3)  # (2, 128, A, FO)
        in_maps.append(
            {
                "inputs": x_pack,
                "mps": np.ascontiguousarray(m).reshape(2, 128, A * FO),
            }
        )
    return in_maps


def kernel(inputs, mps):
    from concourse.bass_utils import run_bass_kernel_spmd

    nc = get_nc()
    res = run_bass_kernel_spmd(
        nc, make_in_maps(inputs, mps), core_ids=list(range(N_CORES))
    )
    return np.array(
        [res.results[c]["out"][0, 0] for c in range(N_CORES)], dtype=np.float32
    )


# revision 25
# speedup vs baseline: 1.1063x; 1.1063x over previous
"""Trainium2 (8-core SPMD) kernel for the ActorCriticTensorNet MPS head.

reference:
    env0 = einsum('e,eoij->oij', x[0], mps[0])
    for a in 1..63: env = sigmoid(env @ einsum('e,eoij->oij', x[a], mps[a]))
    out = einsum('oii->o', env)

Strategy: the computation factorizes perfectly over the output channel o —
the per-agent contractions mat[a][o] and the 63-step sigmoid chain for
channel o never touch any other channel; the channels only meet in the final
trace vector.  So shard by o: core c receives mps[:, :, c, :, :] (32 MB in
bf16) plus the full (tiny) x, computes all 64 mat[g][c] slices locally,
runs its own o=c chain locally, and writes the single scalar out[c]; the
host assembles the 8 scalars.  Zero inter-core communication, and the chain
consumes mats as phase 1 produces them, so its serial latency hides entirely
under the tensor streaming.

Phase-1 layout (per 8-agent block): weight column (gi, jl) reads the
stride-8 single-free-dim AP f = gi*1024 + 8p + jl (p = 4k + j_hi), giving
psum[p, gi*8+jl] = mat[gi][k, j] (k = p>>2, j = (p&3)*8 + jl).  The
psum->stage copy permutes columns (gi,jl)->(jl,gi) so the DRAM block is
d = 64p + jl*8 + gi = k*256 + j_hi*64 + jl*8 + gi: the store, the one
chain-block load (32 x 256, 512B runs) and the per-agent chain weight views
(stride-8, merging to a single free dim) are all clean APs.

Host-side prep packs shards as mps[c] -> (2, 128, 64*1024) [e-chunk, e_low,
(agent, f)] and x -> (2, 128, 64), so every device DMA is contiguous 2 MB.
All inputs are converted to bf16 on the host (PSUM accumulation is fp32;
measured end-to-end relative error ~6e-4 vs the fp32 reference).
"""

import numpy as np

A, E, O, C = 64, 256, 8, 32
FO = C * C  # per-o mat size: 1024
N_CORES = 8
# phase-1/chain segments (start_agent, n_agents): small at the head so the
# PE starts early, small at the tail so the final chain steps hand off at
# fine granularity, big in the middle for DMA efficiency.
SEGS = [(0, 4), (4, 4)] + [(8 * b, 8) for b in range(1, 8)]
HSEG = 4  # stage/chain handoff granularity (agents)

_CACHE = {}


def _build(debug_out=False):
    from concourse import bacc, mybir, tile
    from concourse.masks import make_identity

    F32 = mybir.dt.float32
    BF16 = mybir.dt.bfloat16
    SIG = mybir.ActivationFunctionType.Sigmoid
    nc = bacc.Bacc(
        "TRN2", target_bir_lowering=False, debug=False, num_devices=N_CORES
    )
    x_d = nc.dram_tensor("inputs", [2, 128, A], BF16, kind="ExternalInput")
    mps_d = nc.dram_tensor(
        "mps", [2, 128, A * FO], BF16, kind="ExternalInput"
    )
    out_d = nc.dram_tensor("out", [1, 1], F32, kind="ExternalOutput")
    if debug_out:
        matdbg_d = nc.dram_tensor("matdbg", [A, FO], BF16, kind="ExternalOutput")
        envdbg_d = nc.dram_tensor("envdbg", [C, C], F32, kind="ExternalOutput")

    with tile.TileContext(nc) as tc:
        with (
            tc.tile_pool(name="dram", bufs=1, space="DRAM") as dram,
            tc.tile_pool(name="mps_pool", bufs=9) as mps_pool,
            tc.tile_pool(name="small", bufs=1) as small,
            tc.tile_pool(name="stage_pool", bufs=4) as stage_pool,
            tc.tile_pool(name="chain_pool", bufs=6) as chain_pool,
            tc.tile_pool(name="env_pool", bufs=4) as env_pool,
            tc.tile_pool(name="ps_mat", bufs=4, space="PSUM") as ps_mat,
            tc.tile_pool(name="ps_chain", bufs=3, space="PSUM") as ps_chain,
            tc.tile_pool(name="ps_misc", bufs=1, space="PSUM") as ps_misc,
        ):
            NH = A // HSEG  # number of half-segments
            mat_drams = [
                dram.tile([1, 128 * 8 * HSEG], BF16, name=f"matd{hi}")
                for hi in range(NH)
            ]

            # x_sb[e_lo, eh*64 + g] = x[g, eh*128 + e_lo]
            x_sb = small.tile([128, 2 * A], BF16)
            for eh in range(2):
                nc.sync.dma_start(x_sb[:, eh * A : (eh + 1) * A], x_d[eh])

            ident = small.tile([C, C], BF16)
            make_identity(nc, ident[:])

            envs = [None]


            cvs = {}

            def chain_prefetch(hi):
                # one clean (32, 32*HSEG) load per half-segment; agent gi's
                # weight view [jh: step 8H x4][jl: step H x8] opt-merges
                # into a single free dim (stride HSEG).
                cb = chain_pool.tile(
                    [C, 32 * HSEG], BF16, tag="chain", name=f"cb{hi}"
                )
                nc.gpsimd.dma_start(
                    cb[:],
                    mat_drams[hi][:].rearrange(
                        "u (k r) -> (u k) r", k=C, r=32 * HSEG
                    ),
                )
                cvs[hi] = cb[:].rearrange(
                    "k (jh jl gi) -> k jh jl gi", jh=4, jl=8, gi=HSEG
                )

            def chain_step(g):
                hi = g // HSEG
                cv, gi = cvs[hi], g % HSEG
                init = g == 0
                ps_g = ps_chain.tile([C, C], BF16 if init else F32, tag="cps")
                if init:
                    nc.tensor.transpose(ps_g[:], cv[:, :, :, gi], ident[:])
                else:
                    nc.tensor.matmul(
                        ps_g[:],
                        cv[:, :, :, gi],
                        envs[0][:],
                        start=True,
                        stop=True,
                    )
                env2 = env_pool.tile([C, C], BF16, tag="env")
                if init:
                    nc.vector.tensor_copy(env2[:], ps_g[:])
                else:
                    nc.scalar.activation(env2[:], ps_g[:], SIG)
                envs[0] = env2

            # Software-pipelined emission: mps DMA in 8-agent (2x2MB)
            # segments for bandwidth; stage->DRAM->chain handoff per
            # 4-agent half-segment so chain steps lag only ~LAG agents.
            LAG_AGENTS = 4
            next_chain = 0
            done_half = [False] * (A // HSEG)

            def flush_half(psum_h, hi):
                stage = stage_pool.tile([128, 8 * HSEG], BF16, tag="stage")
                nc.vector.tensor_copy(
                    stage[:].rearrange("p (jl gi) -> p jl gi", jl=8, gi=HSEG),
                    psum_h[:].rearrange("p (gi jl) -> p jl gi", gi=HSEG, jl=8),
                )
                nc.gpsimd.dma_start(mat_drams[hi][:], stage[:])
                chain_prefetch(hi)
                done_half[hi] = True

            for si, (g0, w) in enumerate(SEGS):
                tvs = []
                for eh in range(2):
                    t = mps_pool.tile([128, w * FO], BF16, tag="mps")
                    nc.sync.dma_start(
                        t[:], mps_d[eh, :, g0 * FO : (g0 + w) * FO]
                    )
                    tvs.append(
                        t[:].rearrange(
                            "e (gi p jl) -> e gi p jl", gi=w, p=128, jl=8
                        )
                    )
                psum_h = None
                for gi in range(w):
                    g = g0 + gi
                    if gi % HSEG == 0:
                        psum_h = ps_mat.tile([128, 8 * HSEG], F32, tag="psa")
                    for jl in range(8):
                        for eh in range(2):
                            nc.tensor.matmul(
                                psum_h[
                                    :, (gi % HSEG) * 8 + jl : (gi % HSEG) * 8 + jl + 1
                                ],
                                tvs[eh][:, gi, :, jl],
                                x_sb[:, eh * A + g : eh * A + g + 1],
                                start=(eh == 0),
                                stop=(eh == 1),
                            )
                    if (gi + 1) % HSEG == 0:
                        flush_half(psum_h, g // HSEG)
                    while (
                        next_chain + LAG_AGENTS <= g
                        and done_half[next_chain // HSEG]
                    ):
                        chain_step(next_chain)
                        next_chain += 1
            while next_chain < A:
                chain_step(next_chain)
                next_chain += 1

            env = envs[0]
            if debug_out:
                dbg = small.tile([C, C], F32)
                nc.vector.tensor_copy(dbg[:], env[:])
                nc.sync.dma_start(envdbg_d[:], dbg[:])


            # ---- trace epilogue: out = sum_k env_T[k, k] ----
            masked = small.tile([C, C], BF16)
            nc.gpsimd.affine_select(
                out=masked[:],
                in_=env[:],
                compare_op=mybir.AluOpType.is_equal,
                fill=0.0,
                base=0,
                pattern=[[1, C]],
                channel_multiplier=-1,
            )
            red = small.tile([C, 1], F32)
            nc.vector.tensor_reduce(
                red[:],
                masked[:],
                axis=mybir.AxisListType.X,
                op=mybir.AluOpType.add,
            )
            ones = small.tile([C, 1], F32)
            nc.gpsimd.memset(ones[:], 1.0)
            pt = ps_misc.tile([1, 1], F32)
            nc.tensor.matmul(pt[:], red[:], ones[:], start=True, stop=True)
            osb = small.tile([1, 1], F32)
            nc.vector.tensor_copy(osb[:], pt[:])
            nc.sync.dma_start(out_d[:], osb[:])

    nc.compile()
    return nc


def get_nc():
    if "nc" not in _CACHE:
        _CACHE["nc"] = _build()
    return _CACHE["nc"]


def make_in_maps(inputs, mps):
    import ml_dtypes

    x = np.asarray(inputs, dtype=np.float32).astype(ml_dtypes.bfloat16)
    mps = np.asarray(mps, dtype=np.float32).reshape(A, E, O, FO)
    # x packed as [e_chunk, e_low, agent]
    x_pack = np.ascontiguousarray(x.reshape(A, 2, 128).transpose(1, 2, 0))
    in_maps = []
    for c in range(N_CORES):
        m = mps[:, :, c, :].astype(ml_dtypes.bfloat16)  # (A, E, FO)
        m = m.reshape(A, 2, 128, FO).transpose(1, 2, 0, 3)  # (2, 128, A, FO)
        in_maps.append(
            {
                "inputs": x_pack,
                "mps": np.ascontiguousarray(m).reshape(2, 128, A * FO),
            }
        )
    return in_maps


def kernel(inputs, mps):
    from concourse.bass_utils import run_bass_kernel_spmd

    nc = get_nc()
    res = run_bass_kernel_spmd(
        nc, make_in_maps(inputs, mps), core_ids=list(range(N_CORES))
    )
    return np.array(
        [res.results[c]["out"][0, 0] for c in range(N_CORES)], dtype=np.float32
    )


# revision 26
# speedup vs baseline: 1.1350x; 1.0260x over previous
"""Trainium2 (8-core SPMD) kernel for the ActorCriticTensorNet MPS head.

reference:
    env0 = einsum('e,eoij->oij', x[0], mps[0])
    for a in 1..63: env = sigmoid(env @ einsum('e,eoij->oij', x[a], mps[a]))
    out = einsum('oii->o', env)

Strategy: the computation factorizes perfectly over the output channel o —
the per-agent contractions mat[a][o] and the 63-step sigmoid chain for
channel o never touch any other channel; the channels only meet in the final
trace vector.  So shard by o: core c receives mps[:, :, c, :, :] (32 MB in
bf16) plus the full (tiny) x, computes all 64 mat[g][c] slices locally,
runs its own o=c chain locally, and writes the single scalar out[c]; the
host assembles the 8 scalars.  Zero inter-core communication, and the chain
consumes mats as phase 1 produces them, so its serial latency hides entirely
under the tensor streaming.

Phase-1 layout (per 8-agent block): weight column (gi, jl) reads the
stride-8 single-free-dim AP f = gi*1024 + 8p + jl (p = 4k + j_hi), giving
psum[p, gi*8+jl] = mat[gi][k, j] (k = p>>2, j = (p&3)*8 + jl).  The
psum->stage copy permutes columns (gi,jl)->(jl,gi) so the DRAM block is
d = 64p + jl*8 + gi = k*256 + j_hi*64 + jl*8 + gi: the store, the one
chain-block load (32 x 256, 512B runs) and the per-agent chain weight views
(stride-8, merging to a single free dim) are all clean APs.

Host-side prep packs shards as mps[c] -> (2, 128, 64*1024) [e-chunk, e_low,
(agent, f)] and x -> (2, 128, 64), so every device DMA is contiguous 2 MB.
All inputs are converted to bf16 on the host (PSUM accumulation is fp32;
measured end-to-end relative error ~6e-4 vs the fp32 reference).
"""

import numpy as np

A, E, O, C = 64, 256, 8, 32
FO = C * C  # per-o mat size: 1024
N_CORES = 8
# phase-1/chain segments (start_agent, n_agents): small at the head so the
# PE starts early, small at the tail so the final chain steps hand off at
# fine granularity, big in the middle for DMA efficiency.
SEGS = [(0, 4), (4, 4)] + [(8 * b, 8) for b in range(1, 8)]

_CACHE = {}


def _build(debug_out=False):
    from concourse import bacc, mybir, tile
    from concourse.masks import make_identity

    F32 = mybir.dt.float32
    BF16 = mybir.dt.bfloat16
    SIG = mybir.ActivationFunctionType.Sigmoid
    nc = bacc.Bacc(
        "TRN2", target_bir_lowering=False, debug=False, num_devices=N_CORES
    )
    x_d = nc.dram_tensor("inputs", [2, 128, A], BF16, kind="ExternalInput")
    mps_d = nc.dram_tensor(
        "mps", [2, 128, A * FO], BF16, kind="ExternalInput"
    )
    out_d = nc.dram_tensor("out", [1, 1], F32, kind="ExternalOutput")
    if debug_out:
        matdbg_d = nc.dram_tensor("matdbg", [A, FO], BF16, kind="ExternalOutput")
        envdbg_d = nc.dram_tensor("envdbg", [C, C], F32, kind="ExternalOutput")

    with tile.TileContext(nc) as tc:
        with (
            tc.tile_pool(name="dram", bufs=1, space="DRAM") as dram,
            tc.tile_pool(name="mps_pool", bufs=9) as mps_pool,
            tc.tile_pool(name="small", bufs=1) as small,
            tc.tile_pool(name="stage_pool", bufs=4) as stage_pool,
            tc.tile_pool(name="chain_pool", bufs=6) as chain_pool,
            tc.tile_pool(name="env_pool", bufs=4) as env_pool,
            tc.tile_pool(name="ps_mat", bufs=4, space="PSUM") as ps_mat,
            tc.tile_pool(name="ps_chain", bufs=3, space="PSUM") as ps_chain,
            tc.tile_pool(name="ps_misc", bufs=1, space="PSUM") as ps_misc,
        ):
            seg_of = {}
            for si, (g0, w) in enumerate(SEGS):
                for g in range(g0, g0 + w):
                    seg_of[g] = si
            mat_drams = [
                dram.tile([1, 128 * 8 * w], BF16, name=f"matd{si}")
                for si, (g0, w) in enumerate(SEGS)
            ]

            # x_sb[e_lo, eh*64 + g] = x[g, eh*128 + e_lo]
            x_sb = small.tile([128, 2 * A], BF16)
            for eh in range(2):
                nc.sync.dma_start(x_sb[:, eh * A : (eh + 1) * A], x_d[eh])

            ident = small.tile([C, C], BF16)
            make_identity(nc, ident[:])

            envs = [None]


            cvs = {}

            def chain_prefetch(si, w):
                # one clean (32, 32w) load per segment; agent gi's weight
                # view [jh: step 8w x4][jl: step w x8] opt-merges into a
                # single free dim (stride w).
                cb = chain_pool.tile(
                    [C, 32 * w], BF16, tag="chain", name=f"cb{si}"
                )
                nc.gpsimd.dma_start(
                    cb[:],
                    mat_drams[si][:].rearrange(
                        "u (k r) -> (u k) r", k=C, r=32 * w
                    ),
                )
                cvs[si] = cb[:].rearrange(
                    "k (jh jl gi) -> k jh jl gi", jh=4, jl=8, gi=w
                )

            def chain_step(g):
                si = seg_of[g]
                cv, gi = cvs[si], g - SEGS[si][0]
                init = g == 0
                ps_g = ps_chain.tile([C, C], BF16 if init else F32, tag="cps")
                if init:
                    nc.tensor.transpose(ps_g[:], cv[:, :, :, gi], ident[:])
                else:
                    nc.tensor.matmul(
                        ps_g[:],
                        cv[:, :, :, gi],
                        envs[0][:],
                        start=True,
                        stop=True,
                    )
                env2 = env_pool.tile([C, C], BF16, tag="env")
                if init:
                    nc.vector.tensor_copy(env2[:], ps_g[:])
                else:
                    nc.scalar.activation(env2[:], ps_g[:], SIG)
                envs[0] = env2

            # Software-pipelined emission: chain steps lag ~8 agents behind
            # phase 1 so each chain matmul's sigmoid dependency has retired
            # by the time the in-order PE reaches it.
            LAG_AGENTS = 8
            next_chain = 0

            for si, (g0, w) in enumerate(SEGS):
                psum_b = ps_mat.tile([128, 8 * w], F32, tag="psa")
                tvs = []
                for eh in range(2):
                    t = mps_pool.tile([128, w * FO], BF16, tag="mps")
                    nc.sync.dma_start(
                        t[:], mps_d[eh, :, g0 * FO : (g0 + w) * FO]
                    )
                    tvs.append(
                        t[:].rearrange(
                            "e (gi p jl) -> e gi p jl", gi=w, p=128, jl=8
                        )
                    )
                for gi in range(w):
                    g = g0 + gi
                    for jl in range(8):
                        for eh in range(2):
                            nc.tensor.matmul(
                                psum_b[:, gi * 8 + jl : gi * 8 + jl + 1],
                                tvs[eh][:, gi, :, jl],
                                x_sb[:, eh * A + g : eh * A + g + 1],
                                start=(eh == 0),
                                stop=(eh == 1),
                            )
                    while next_chain + LAG_AGENTS <= g and seg_of[
                        next_chain
                    ] < si:
                        chain_step(next_chain)
                        next_chain += 1
                stage = stage_pool.tile([128, 8 * w], BF16, tag="stage")
                nc.vector.tensor_copy(
                    stage[:].rearrange("p (jl gi) -> p jl gi", jl=8, gi=w),
                    psum_b[:].rearrange("p (gi jl) -> p jl gi", gi=w, jl=8),
                )
                nc.gpsimd.dma_start(mat_drams[si][:], stage[:])
                chain_prefetch(si, w)
            while next_chain < A:
                chain_step(next_chain)
                next_chain += 1

            env = envs[0]
            if debug_out:
                dbg = small.tile([C, C], F32)
                nc.vector.tensor_copy(dbg[:], env[:])
                nc.sync.dma_start(envdbg_d[:], dbg[:])


            # ---- trace epilogue: out = sum_k env_T[k, k] ----
            masked = small.tile([C, C], BF16)
            nc.gpsimd.affine_select(
                out=masked[:],
                in_=env[:],
                compare_op=mybir.AluOpType.is_equal,
                fill=0.0,
                base=0,
                pattern=[[1, C]],
                channel_multiplier=-1,
            )
            red = small.tile([C, 1], F32)
            nc.vector.tensor_reduce(
                red[:],
                masked[:],
                axis=mybir.AxisListType.X,
                op=mybir.AluOpType.add,
            )
            ones = small.tile([C, 1], F32)
            nc.gpsimd.memset(ones[:], 1.0)
            pt = ps_misc.tile([1, 1], F32)
            nc.tensor.matmul(pt[:], red[:], ones[:], start=True, stop=True)
            osb = small.tile([1, 1], F32)
            nc.vector.tensor_copy(osb[:], pt[:])
            nc.sync.dma_start(out_d[:], osb[:])

    nc.compile()
    return nc


def get_nc():
    if "nc" not in _CACHE:
        _CACHE["nc"] = _build()
    return _CACHE["nc"]


def make_in_maps(inputs, mps):
    import ml_dtypes

    x = np.asarray(inputs, dtype=np.float32).astype(ml_dtypes.bfloat16)
    mps = np.asarray(mps, dtype=np.float32).reshape(A, E, O, FO)
    # x packed as [e_chunk, e_low, agent]
    x_pack = np.ascontiguousarray(x.reshape(A, 2, 128).transpose(1, 2, 0))
    in_maps = []
    for c in range(N_CORES):
        m = mps[:, :, c, :].astype(ml_dtypes.bfloat16)  # (A, E, FO)
        m = m.reshape(A, 2, 128, FO).transpose(1, 2, 0, 3)  # (2, 128, A, FO)
        in_maps.append(
            {
                "inputs": x_pack,
                "mps": np.ascontiguousarray(m).reshape(2, 128, A * FO),
            }
        )
    return in_maps


def kernel(inputs, mps):
    from concourse.bass_utils import run_bass_kernel_spmd

    nc = get_nc()
    in_maps = make_in_maps(inputs, mps)
    try:
        res = run_bass_kernel_spmd(nc, in_maps, core_ids=list(range(N_CORES)))
    except Exception:
        # rare transient NRT failures; one retry
        res = run_bass_kernel_spmd(nc, in_maps, core_ids=list(range(N_CORES)))
    return np.array(
        [res.results[c]["out"][0, 0] for c in range(N_CORES)], dtype=np.float32
    )
